# revision 14
# baseline (speedup 1.0000x reference)
"""Trainium2 Bass kernel for nn_ConvFCLIFNet.

Pipeline: x_seq (T=64, B=512, 1, 28, 28) -> conv2x2(valid) -> FC(729) -> LIF
scan over T -> spike sequence (T, B, 729) in {0.0, 1.0}.

Strategy
--------
- conv + FC + bias + 1/tau fold into ONE matmul: y*0.5 = x_aug @ W_aug where
  x_aug = [x_pixels(784), 1.0] and W_aug[p, o] = 0.5 * (fc_w @ C)^T (C = conv
  scatter), bias row at p=784. Rows 785..895 of W_aug are zero, so the kt=6
  k-chunk can run with all 128 partitions (garbage x rows * 0 weight = 0).
- Data-parallel over 8 NeuronCores: 64 samples each.
- Matmul: W chunks stationary [128 pixels, 128 features] (f32r), x^T moving
  [128 pixels, G*64 samples] with G=8 (moving 512 >= 256 keeps f32r at the
  full 1 cycle/row PE rate).
- PSUM is j-split into two halves ps_a (features 0..383) / ps_b (384..767),
  6KB each, so the LIF drain of one half overlaps matmuls into the other:
  the tensor engine never idles and stays at the 2.4GHz hot p-state.
- LIF scan: custom DVE op per (timestep, half):
      u = (q_prev == SENT) ? 0 : q_prev;  w = z + u
      q = (w >= 1) ? SENT : 0.5 * w
  Spike decode on ScalarE: s = Relu(q + (1 - SENT)) -> exactly 1.0 iff spiked,
  emitted as uint8 into a per-group staging tile (one 3KB/partition store per
  group instead of 64 f32 stores).
- Host does only layout staging (shard, pixel-major transpose, weight fold)
  plus the final gather/decode.
"""
import numpy as np

import concourse.bacc as bacc
import concourse.mybir as mybir
import concourse.tile as tile
from concourse.bass_utils import run_bass_kernel_spmd

# ---------------------------------------------------------------- constants
T, B, H, W = 64, 512, 28, 28
NPIX = H * W            # 784
NF = 729                # fc features
NCORES = 8
BS = B // NCORES        # 64 samples per core
G = 8                   # timesteps per matmul group
NG = T // G             # 8 groups
NJ = 6                  # feature chunks of 128 (768 padded)
NJC = 2                 # feature chunks per PSUM third
KT = 7                  # contraction k-tiles of 128 (785 real rows + zero pad)
NS = G * BS             # moving free size = 512
SENT = float(2 ** 20)

_CACHE = {}

# ------------------------------------------------------------ custom DVE op

def _register_lif_op():
    from concourse.dve_spec import Spec, Src0, Src1, C0, C1, Zero, One, select, eq, lower
    from concourse.dve_uop import DveOpSpec
    from concourse import dve_ops

    name = "LIF_STEP_ANT"
    for op in dve_ops.OPS:
        if op.name == name:
            return op

    def _ref(in0, in1, s0, s1, imm2=None):
        u = np.where(in1 == s0, 0.0, in1).astype(np.float32)
        w = (in0 + u).astype(np.float32)
        return np.where(w >= 1.0, np.float32(s0), (w * np.float32(s1)).astype(np.float32))

    _u = select(eq(Src1, C0), Zero, Src1)
    _w = Src0 + _u
    spec = Spec(body=select(_w >= One, C0, _w * C1), reference=_ref)

    row = dve_ops._CUSTOM_DVE_ROW_BASE + len(dve_ops.OPS)
    assert row < 0x20
    dve_ops._SUB_OPCODE_FOR_NAME[name] = row
    shas = {}
    for ver in ("v3", "v4"):
        s = DveOpSpec(name=name, opcode=row, uops=lower(spec, ver=ver), rd1_en=True)
        shas[ver] = s.sha(ver)
    op = dve_ops.DveOp(name, spec, subdim=False, uops_sha=shas)
    dve_ops.OPS.append(op)
    dve_ops.CUSTOM_DVE_SPECS[name] = spec
    return op

# ------------------------------------------------------------- device build

def _build():
    lif = _register_lif_op()
    nc = bacc.Bacc(None, target_bir_lowering=False, debug=False)
    f32, f32r, u8 = mybir.dt.float32, mybir.dt.float32r, mybir.dt.uint8
    with tile.TileContext(nc) as tc:
        with tc.tile_pool(name="dram", bufs=1, space="DRAM") as dram, \
             tc.tile_pool(name="consts", bufs=1) as consts, \
             tc.tile_pool(name="xpool", bufs=3) as xpool, \
             tc.tile_pool(name="qpool", bufs=2) as qpool, \
             tc.tile_pool(name="spool", bufs=2) as spool, \
             tc.tile_pool(name="wmps", bufs=1, space="PSUM") as wmps_pool, \
             tc.tile_pool(name="ps0", bufs=1, space="PSUM") as ps0_pool, \
             tc.tile_pool(name="ps1", bufs=1, space="PSUM") as ps1_pool, \
             tc.tile_pool(name="ps2", bufs=1, space="PSUM") as ps2_pool:
            ps_pools = [ps0_pool, ps1_pool, ps2_pool]
            x_in = dram.tile([NG, 128, KT, NS], f32r, kind="ExternalInput",
                             name="x_in", uniquify=False)
            # w_in: pre-permuted on host to [p][j][kt][m] (j-major so the
            # first output column's weights arrive in one small DMA)
            w_in = dram.tile([128, NJ, KT, 128], f32r, kind="ExternalInput",
                             name="w_in", uniquify=False)
            # out: partition-major u8 spikes, [group][p][chunk][tl][jc][sample]
            out = dram.tile([NG, 128, 3, G, NJC, BS], u8, kind="ExternalOutput",
                            name="out", uniquify=False)

            wsb = consts.tile([128, NJ, KT, 128], f32r)
            bias_t = consts.tile([128, 1], f32)
            nc.vector.memset(bias_t[:, :], float(1.0 - SENT))

            # PE-clock warmup: a few throwaway matmuls on const-fed scratch
            # keep the tensor engine busy through its p-state ramp while the
            # real inputs stream in.
            wm_w = consts.tile([128, 128], f32r)
            wm_x = consts.tile([128, 512], f32r)
            nc.sync.dma_start(out=wm_w[:, :], in_=w_in[:, 0, 0, :])
            nc.sync.dma_start(out=wm_x[:, :],
                              in_=w_in[:, 0, 1:5, :].rearrange("p k m -> p (k m)"))
            wm_ps = wmps_pool.tile([128, 512], f32)
            for _ in range(12):
                nc.tensor.matmul(wm_ps[:, :], lhsT=wm_w[:, :], rhs=wm_x[:, :],
                                 start=True, stop=True)

            # g=0 prologue: x chunks (gpsimd queue) and j-major weight slices
            # (sync queue) issue in parallel; later groups prefetch under
            # compute anyway.
            x_sb0 = xpool.tile([128, KT, NS], f32r, name="x_sb", tag="x")
            nc.gpsimd.dma_start(out=x_sb0[:, 0, :], in_=x_in[0, :, 0, :])
            nc.sync.dma_start(out=wsb[:, 0, :, :], in_=w_in[:, 0, :, :])
            for kt in range(1, KT):
                nc.gpsimd.dma_start(out=x_sb0[:, kt, :], in_=x_in[0, :, kt, :])
            for j in range(1, NJ):
                nc.sync.dma_start(out=wsb[:, j, :, :], in_=w_in[:, j, :, :])

            # per-chunk q history [128, tl, jc, sample]; slice G-1 of the
            # previous group's tile seeds the LIF chain (memset -> v0 = 0).
            qs = []
            for c in range(3):
                q = qpool.tile([128, G, NJC, BS], f32, name=f"q{c}", tag=f"q{c}")
                nc.vector.memset(q[:, :, :, :], 0.0)
                qs.append(q)

            for g in range(NG):
                if g == 0:
                    x_sb = x_sb0
                else:
                    x_sb = xpool.tile([128, KT, NS], f32r, name="x_sb", tag="x")
                    # one DMA per group: 14KB contiguous per partition (rows
                    # 785..895 are zero-padded host-side)
                    nc.gpsimd.dma_start(out=x_sb[:, :, :], in_=x_in[g])

                pss = [ps_pools[c].tile([128, NJC, NS], f32, name=f"ps{c}",
                                        tag=f"ps{c}") for c in range(3)]
                for j in range(NJ):
                    c, jj = divmod(j, NJC)
                    for kt in range(KT):
                        nc.tensor.matmul(
                            pss[c][:, jj, :],
                            lhsT=wsb[:, j, kt, :],
                            rhs=x_sb[:, kt, :],
                            start=(kt == 0), stop=(kt == KT - 1),
                        )

                q2s = [qpool.tile([128, G, NJC, BS], f32, name=f"q{c}",
                                  tag=f"q{c}") for c in range(3)]
                for tl in range(G):
                    for c in range(3):
                        nc.vector._custom_dve(
                            lif,
                            out=q2s[c][:, tl, :, :],
                            in0=pss[c][:, :, tl * BS:(tl + 1) * BS],
                            in1=(qs[c][:, G - 1, :, :] if tl == 0
                                 else q2s[c][:, tl - 1, :, :]),
                            s0=SENT, s1=0.5,
                        )
                # one spike decode + store per chunk; early chunks ship while
                # later chains are still draining
                for c in range(3):
                    s_c = spool.tile([128, G, NJC, BS], u8, name=f"s{c}",
                                     tag=f"s{c}")
                    nc.scalar.activation(
                        s_c[:, :, :, :], q2s[c][:, :, :, :],
                        mybir.ActivationFunctionType.Relu,
                        bias=bias_t[:, :], scale=1.0,
                    )
                    nc.sync.dma_start(out=out[g, :, c], in_=s_c[:, :, :, :])
                qs = q2s
    nc.compile()
    return nc

# --------------------------------------------------------------- host side

def _prep_weights(conv_w, fc_w, fc_b):
    """W_aug permuted to [128 p, KT, NJ, 128 m]: row kt*128+p = pixel (or bias
    at 784, zero pad above), col j*128+m = feature; scaled by 0.5 (tau fold)."""
    cw = conv_w.reshape(2, 2).astype(np.float32)
    fcw = fc_w.astype(np.float32).reshape(NF, 27, 27)
    tmp = np.zeros((NF, H, W), np.float32)
    for dr in range(2):
        for dc in range(2):
            tmp[:, dr:dr + 27, dc:dc + 27] += cw[dr, dc] * fcw
    w_eff = tmp.reshape(NF, NPIX)                     # [729, 784]
    w_aug = np.zeros((KT * 128, NJ * 128), np.float32)
    w_aug[:NPIX, :NF] = 0.5 * w_eff.T
    w_aug[NPIX, :NF] = 0.5 * fc_b.astype(np.float32)
    # [KT*128, NJ*128] -> [KT, 128, NJ, 128] -> [128, NJ, KT, 128]
    return np.ascontiguousarray(
        w_aug.reshape(KT, 128, NJ, 128).transpose(1, 2, 0, 3))

def _prep_x(x_seq):
    """Per-core pixel-major inputs [NCORES][NG, 128, KT, NS]."""
    xs = np.ascontiguousarray(x_seq.reshape(T, NCORES, BS, NPIX))
    # -> [core, group, pixel, (tl, sample)]
    xt = xs.transpose(1, 0, 3, 2).reshape(NCORES, NG, G, NPIX, BS)
    xt = xt.transpose(0, 1, 3, 2, 4).reshape(NCORES, NG, NPIX, NS)
    xp = np.zeros((NCORES, NG, KT * 128, NS), np.float32)
    xp[:, :, :NPIX, :] = xt
    xp[:, :, NPIX, :] = 1.0
    # rows -> [128, KT]: row kt*128+p at [p, kt]
    xp = xp.reshape(NCORES, NG, KT, 128, NS).transpose(0, 1, 3, 2, 4)
    return np.ascontiguousarray(xp)

def kernel(x_seq, conv_w, fc_w, fc_b):
    if "nc" not in _CACHE:
        _CACHE["nc"] = _build()
    nc = _CACHE["nc"]
    w_aug = _prep_weights(conv_w, fc_w, fc_b)
    xp = _prep_x(np.asarray(x_seq, dtype=np.float32))
    in_maps = [{"x_in": np.ascontiguousarray(xp[c]), "w_in": w_aug}
               for c in range(NCORES)]
    res = run_bass_kernel_spmd(nc, in_maps, core_ids=list(range(NCORES)))
    _CACHE["last_result"] = res
    full = np.empty((T, B, NF), np.float32)
    for c in range(NCORES):
        o = res.results[c]["out"]             # [NG, 128, 2, G, NJH, BS] u8
        # (g, p, h, tl, jh, s) -> (g, tl, s, h, jh, p); f = (h*NJH+jh)*128+p
        full[:, c * BS:(c + 1) * BS, :] = (
            o.transpose(0, 3, 5, 2, 4, 1).reshape(T, BS, NJ * 128)[:, :, :NF]
            .astype(np.float32))
    return full
